# revision 9
# baseline (speedup 1.0000x reference)
"""MDN-RNN loss kernel v4: transposed layout, PE-driven reductions.

Layout (per core, R=2048 rows): host ships tensors TRANSPOSED so the
feature dim D sits on partitions and the token rows sit on the free dim:
    tgtT  [D, R]        bf16   (chunks c: [128, R] x8 + [64, R])
    meanT [D, K, R]     bf16   (chunk c rows c*128.., k middle)
    lstdT [D, K, R]     fp8e4
    lmx   [P, T*K]      f32    (row-major packing for the logsumexp tail)

Per chunk c (free size 2048 per k-slice):
    ACT:  e1 = exp(-lstd) (one [Pc,K,R] pass), Square for ~55% of k-slices
    DVE:  diff_k = tgt - mean_k (per-k tt, bf16 2x), z = diff*e1 (3D tt),
          w = z*z for the remaining k-slices
    PE :  h_k   += ones^T @ w_k   (accumulates over c into PSUM [K, R])
          sls_k += ones^T @ lstd_k
Tail: score1 = -0.5*h - sls (DVE stt from PSUM), PE-transpose score1 back
to row-major [128, T, K], + log_mix, then the standard stable logsumexp
smalls and the [P,1] partial-sum output.
"""

import sys

if "/opt/trn_rl_repo" not in sys.path:
    sys.path.insert(0, "/opt/trn_rl_repo")

import numpy as np
import ml_dtypes

N = 16384
K = 5
D = 1088
KD = K * D
NCORES = 8
R = N // NCORES          # 2048 rows per core
P = 128                  # partitions
T = R // P               # 16 row-tiles (tail packing)
NC_FULL = D // P         # 8 full chunks
TAILP = D - NC_FULL * P  # 64
NCHUNK = NC_FULL + 1     # 9

# per-chunk count of k-slices whose square runs on ACT (rest: DVE w=z*z)
ACT_SQ = {c: 2 for c in range(NCHUNK)}

_NC = None


def _build():
    import concourse.bacc as bacc
    import concourse.bass as bass
    import concourse.tile as tile
    from concourse import mybir

    AF = mybir.ActivationFunctionType
    AL = mybir.AluOpType
    AX = mybir.AxisListType
    f32 = mybir.dt.float32
    bf16 = mybir.dt.bfloat16
    f8 = mybir.dt.float8e4

    nc = bacc.Bacc("TRN2", debug=False)
    tgt = nc.dram_tensor("tgt", [D, R], bf16, kind="ExternalInput").ap()
    mean = nc.dram_tensor("mean", [D, K, R], bf16, kind="ExternalInput").ap()
    lstd = nc.dram_tensor("lstd", [D, K, R], f8, kind="ExternalInput").ap()
    lmx = nc.dram_tensor("lmx", [P, T * K], f32, kind="ExternalInput").ap()
    ident_d = nc.dram_tensor("ident", [K, K], f32, kind="ExternalInput").ap()
    out = nc.dram_tensor("partial", [P, 1], f32, kind="ExternalOutput").ap()

    with tile.TileContext(nc) as tc:
        with (
            tc.tile_pool(name="tgt_p", bufs=1) as tgt_p,
            tc.tile_pool(name="mean_p", bufs=2) as mean_p,
            tc.tile_pool(name="lstd_p", bufs=4) as lstd_p,
            tc.tile_pool(name="e1_p", bufs=2) as e1_p,
            tc.tile_pool(name="ones_p", bufs=1) as ones_p,
            tc.tile_pool(name="small_p", bufs=2) as small_p,
            tc.tile_pool(name="persist", bufs=1) as persist,
            tc.tile_pool(name="psum_h", bufs=1, space="PSUM") as psum_h,
            tc.tile_pool(name="psum_s", bufs=1, space="PSUM") as psum_s,
        ):
            t_lmx = persist.tile([P, T * K], f32)
            nc.sync.dma_start(out=t_lmx, in_=lmx)

            ones_bf = ones_p.tile([P, 1], bf16)
            nc.vector.memset(ones_bf, 1.0)
            twos_f8 = ones_p.tile([P, 1], f8)
            nc.vector.memset(twos_f8, 2.0)
            # 5x5 identity for the PE transpose tail (memset can't write at
            # partition bases > 0, so DMA it in)
            ident = ones_p.tile([K, K], f32)
            nc.sync.dma_start(out=ident, in_=ident_d)

            # whole-core resident target (36 KB/partition)
            t_tgt = tgt_p.tile([P, NCHUNK, R], bf16)

            # merged accumulators v_k = sum_d z^2 + 2*sum_d logstd, one PSUM
            # row per k at matmul-legal base partitions {0,32,64,96} + {0}
            vA = psum_h.tile([P, R], f32)       # k=0,1,2 at partitions 0/32/64
            vB = psum_s.tile([P, R], f32)       # k=3,4 at partitions 0/32

            def v_row(k):
                if k < 3:
                    return vA[k * 32 : k * 32 + 1, :]
                return vB[(k - 3) * 32 : (k - 3) * 32 + 1, :]

            state = {}
            pending = []   # queued (lstd_tile, chunk, k) sls matmul groups

            def pc_of(c):
                return P if c < NC_FULL else TAILP

            lstate = {}

            def emit_sls(c):
                """DMA logstd + its 20 PE matmuls: runs chunks ahead of the
                main stage so the PE queue never drains (keeps the tensor
                engine past its 3us continuous-execution ramp threshold)."""
                pc = pc_of(c)
                rows = slice(c * P, c * P + pc)
                t_lstd = lstd_p.tile([P, K, R], f8)
                t_mean = mean_p.tile([P, K, R], bf16)
                if c == 0:
                    # chunked startup: first sls matmuls + exp fire after
                    # ~1/5 of the load instead of the whole 1.25 MB
                    for k in range(K):
                        nc.sync.dma_start(
                            out=t_lstd[:pc, k, :], in_=lstd[rows][:, k, :]
                        )
                    nc.sync.dma_start(
                        out=t_tgt[:pc, 0, :], in_=tgt[0:pc]
                    )
                    for k in range(K):
                        nc.sync.dma_start(
                            out=t_mean[:pc, k, :], in_=mean[rows][:, k, :]
                        )
                    for cc in range(1, NCHUNK):
                        pcc = pc_of(cc)
                        nc.sync.dma_start(
                            out=t_tgt[:pcc, cc, :],
                            in_=tgt[cc * P : cc * P + pcc],
                        )
                else:
                    nc.sync.dma_start(out=t_lstd[:pc], in_=lstd[rows])
                    nc.sync.dma_start(out=t_mean[:pc], in_=mean[rows])
                # 2*sum(logstd) matmuls are queued (not emitted): drained at
                # a fixed rate between w-matmul groups so PE keeps ready,
                # non-blocking work through the late chunks' stall windows
                for k in range(K):
                    pending.append((t_lstd, c, k))
                lstate[c] = (t_lstd, t_mean)

            def emit_sls_mm(t_lstd, c2, k):
                pcx = pc_of(c2)
                vr = v_row(k)
                for b in range(0, R, 512):
                    nc.tensor.matmul(
                        vr[:, b : b + 512],
                        twos_f8[:pcx],
                        t_lstd[:pcx, k, b : b + 512],
                        start=(c2 == 0),
                        stop=False,
                        skip_group_check=True,
                    )

            def emit_a(c):
                pc = pc_of(c)
                t_lstd, t_mean = lstate.pop(c)

                # e1 = exp(-lstd), one 3D pass on ACT
                t_e1 = e1_p.tile([P, K, R], bf16)
                nc.scalar.activation(
                    out=t_e1[:pc], in_=t_lstd[:pc], func=AF.Exp, scale=-1.0
                )
                # diff_k = tgt_c - mean_k, per-k 2D tt (keeps bf16 2x mode)
                for k in range(K):
                    nc.vector.tensor_tensor(
                        out=t_mean[:pc, k, :], in0=t_tgt[:pc, c, :],
                        in1=t_mean[:pc, k, :], op=AL.subtract,
                    )
                state[c] = (t_mean, t_e1)

            def emit_b(c):
                pc = pc_of(c)
                t_mean, t_e1 = state.pop(c)
                a_sq = ACT_SQ[c]
                if c == NCHUNK - 1:
                    # all sls matmuls must precede this chunk's stop=True
                    # w-matmuls of their PSUM region
                    while pending:
                        emit_sls_mm(*pending.pop(0))
                budget = 4
                for k in range(K):
                    # per-k z so the first square (and its PE matmuls)
                    # releases after ~1.2us instead of after the whole 3D z
                    nc.vector.tensor_tensor(
                        out=t_mean[:pc, k, :], in0=t_mean[:pc, k, :],
                        in1=t_e1[:pc, k, :], op=AL.mult,
                    )
                    zk = t_mean[:pc, k, :]
                    wk = t_e1[:pc, k, :]      # e1 slice is dead after z
                    if k < a_sq:
                        nc.scalar.activation(out=wk, in_=zk, func=AF.Square)
                    else:
                        nc.vector.tensor_tensor(out=wk, in0=zk, in1=zk, op=AL.mult)
                    vr = v_row(k)
                    for b in range(0, R, 512):
                        nc.tensor.matmul(
                            vr[:, b : b + 512],
                            ones_bf[:pc],
                            wk[:, b : b + 512],
                            start=False,
                            stop=(c == NCHUNK - 1),
                            skip_group_check=True,
                        )
                    if budget and pending:
                        emit_sls_mm(*pending.pop(0))
                        budget -= 1

            # exp(c+1) is emitted AFTER emit_b(c) so the ACT queue never
            # head-of-line-blocks chunk c's squares behind the next exp
            emit_sls(0)
            emit_sls(1)
            emit_sls(2)
            # prime PE with the first two chunks' worth of sls matmuls
            for _ in range(2 * K):
                emit_sls_mm(*pending.pop(0))
            emit_a(0)
            for c in range(NCHUNK):
                if c + 3 < NCHUNK:
                    emit_sls(c + 3)
                emit_b(c)
                if c + 1 < NCHUNK:
                    emit_a(c + 1)

            # ---- tail ----
            # PSUM -> SBUF stage (ACT, partition-aligned), then repack the
            # five rows onto partitions 0..4 with single-row SBUF DMAs
            t_stage = persist.tile([P, 2, R], f32)
            for k in range(K):
                a, b = (k, 0) if k < 3 else (k - 3, 1)
                dst = t_stage[a * 32 : a * 32 + 1, b, :]
                nc.scalar.activation(out=dst, in_=v_row(k), func=AF.Copy)
            t_sc1 = persist.tile([K, R], f32)
            for k in range(K):
                a, b = (k, 0) if k < 3 else (k - 3, 1)
                nc.sync.dma_start(
                    out=t_sc1[k : k + 1, :],
                    in_=t_stage[a * 32 : a * 32 + 1, b, :],
                )
            # transpose [K, R] -> row-major [128, T, K] via 16 PE transposes,
            # reusing vA's (now dead) PSUM banks as the destination
            sc_ps = vA[:, 0 : T * K].rearrange("p (t k) -> p t k", k=K)
            for t in range(T):
                nc.tensor.transpose(
                    sc_ps[:, t, :], t_sc1[:, t * P : (t + 1) * P], ident
                )
            # score = -0.5*v + log_mix (row-major packing)
            t_sc2 = persist.tile([P, T, K], f32)
            nc.vector.scalar_tensor_tensor(
                out=t_sc2, in0=sc_ps, scalar=-0.5,
                in1=t_lmx.rearrange("p (t k) -> p t k", k=K),
                op0=AL.mult, op1=AL.add,
            )
            t_nm = persist.tile([P, T], f32)
            nc.vector.tensor_reduce(
                out=t_nm, in_=t_sc2, axis=AX.X, op=AL.max, negate=True
            )
            nm_b = bass.AP(
                tensor=t_nm.tensor, offset=t_nm.offset,
                ap=[t_nm.ap[0], t_nm.ap[1], [0, K]],
            )
            t_es = persist.tile([P, T, K], f32)
            nc.vector.tensor_tensor(out=t_es, in0=t_sc2, in1=nm_b, op=AL.add)
            t_ex = persist.tile([P, T, K], f32)
            nc.scalar.activation(out=t_ex, in_=t_es, func=AF.Exp)
            t_S = persist.tile([P, T], f32)
            nc.vector.tensor_reduce(out=t_S, in_=t_ex, axis=AX.X, op=AL.add)
            t_lns = persist.tile([P, T], f32)
            nc.scalar.activation(out=t_lns, in_=t_S, func=AF.Ln)
            t_accv = persist.tile([P, T], f32)
            nc.vector.tensor_tensor(out=t_accv, in0=t_nm, in1=t_lns, op=AL.subtract)
            t_tot = persist.tile([P, 1], f32)
            nc.vector.tensor_reduce(out=t_tot, in_=t_accv, axis=AX.X, op=AL.add)
            nc.sync.dma_start(out=out, in_=t_tot)

    nc.compile()
    return nc


def get_nc():
    global _NC
    if _NC is None:
        _NC = _build()
    return _NC


def make_in_maps(target, s_mean, s_logstd, log_mix_coeffs):
    target = np.asarray(target, dtype=np.float32)
    s_mean = np.asarray(s_mean, dtype=np.float32)
    s_logstd = np.asarray(s_logstd, dtype=np.float32)
    lm = np.asarray(log_mix_coeffs, dtype=np.float32)
    in_maps = []
    for c in range(NCORES):
        rows = slice(c * R, (c + 1) * R)
        tgtT = np.ascontiguousarray(target[rows].T.astype(ml_dtypes.bfloat16))
        meanT = np.ascontiguousarray(
            s_mean[rows].reshape(R, K, D).transpose(2, 1, 0)
            .astype(ml_dtypes.bfloat16)
        )
        lstdT = np.ascontiguousarray(
            s_logstd[rows].reshape(R, K, D).transpose(2, 1, 0)
            .astype(ml_dtypes.float8_e4m3)
        )
        lmx = np.ascontiguousarray(
            lm[rows].reshape(T, P, K).transpose(1, 0, 2).reshape(P, T * K)
        )
        in_maps.append({
            "tgt": tgtT, "mean": meanT, "lstd": lstdT, "lmx": lmx,
            "ident": np.eye(K, dtype=np.float32),
        })
    return in_maps


def combine(results):
    total = sum(float(np.asarray(r["partial"], dtype=np.float64).sum()) for r in results)
    return np.float32(total / N)


def kernel(target, s_mean, s_logstd, log_mix_coeffs):
    from concourse.bass_utils import run_bass_kernel_spmd

    nc = get_nc()
    in_maps = make_in_maps(target, s_mean, s_logstd, log_mix_coeffs)
    res = run_bass_kernel_spmd(nc, in_maps, core_ids=list(range(NCORES)))
    return combine(res.results)
